# revision 2
# baseline (speedup 1.0000x reference)
"""LEM cell (ODE2) Bass kernel for Trainium2, 8-core data-parallel, fp8 GEMMs.

Math (per batch row b):
  ti = x @ W_ih.T + b_ih                  # [B, 4H]
  th = y @ W_hh.T + b_hh                  # [B, 3H]
  tdt = dt @ W_dt.T + b_dt                # [B, 2]
  ms_dt_bar = sig(tdt[:,0]) * sig(ti[:, :H]   + th[:, :H])
  ms_dt     = sig(tdt[:,1]) * sig(ti[:, H:2H] + th[:, H:2H])
  z_new = (1-ms_dt) * z + ms_dt * tanh(ti[:, 3H:] + th[:, 2H:3H])
  y_new = (1-ms_dt_bar) * y + ms_dt_bar * tanh(z_new @ W_z.T + b_z + ti[:, 2H:3H])
  returns (y_new, z_new)

Strategy: batch sharded across 8 cores (2048 rows each), feature-major on
chip. All GEMMs run as fp8e4m3 DoubleRow matmuls (256-deep contraction per
instruction, 2x fp32r rate). Weights are pre-scaled by S=256 so they sit in
e4m3's normal range; the PSUM descale (1/S) rides the activation scale. The
y/z tensors used in the elementwise combines are bf16 copies; outputs fp32.
The i+h sums and the i_z + z_new@W_z.T sum come free by accumulating both
halves into the same PSUM bank.
"""

import sys

_REPO = "/opt/trn_rl_repo"
if _REPO not in sys.path:
    sys.path.insert(0, _REPO)

from contextlib import ExitStack

import numpy as np
import ml_dtypes

import concourse.bacc as bacc
import concourse.tile as tile
from concourse import mybir
from concourse.bass_utils import run_bass_kernel_spmd

P = 128
F32 = mybir.dt.float32
BF16 = mybir.dt.bfloat16
F8 = mybir.dt.float8e4
AF = mybir.ActivationFunctionType
DR = mybir.MatmulPerfMode.DoubleRow

E4NP = ml_dtypes.float8_e4m3
BFNP = ml_dtypes.bfloat16

N_CORES = 8
NINP = 1024
NHID = 1024
BATCH = 16384
WSCALE = 256.0

LAST_RESULTS = None  # BassKernelResults of the most recent kernel() call


def build_nc(
    K,            # input feature dim (x)
    H,            # hidden dim (y/z)
    B_shard,      # batch rows per core (single resident panel)
    chunk,        # matmul moving-dim size (<=512 fp32 psum)
    wdt00, wdt10,  # W_dt scalars (baked immediates; b_dt rides in biasP)
    w_bufs=8,
    ps_bufs=8,
):
    NJT = H // P            # output feature tiles per H-sized group
    NKT2 = K // (2 * P)     # x contraction pair-tiles
    NHT2 = H // (2 * P)     # y/z contraction pair-tiles
    NPAIR = NKT2 + NHT2
    nch = B_shard // chunk
    descale = 1.0 / WSCALE

    nc = bacc.Bacc(trn_type="TRN2", target_bir_lowering=False)

    # chunk-blocked fp8 moving operands: [kt2, c, p, i*chunk + b] so each
    # (kt2, c) load is 128 descriptors of contiguous 1KB
    xT8 = nc.declare_dram_parameter("xT8", [NKT2, nch, P, 2 * chunk], F8, isOutput=False)
    yT8 = nc.declare_dram_parameter("yT8", [NHT2, nch, P, 2 * chunk], F8, isOutput=False)
    yTb = nc.declare_dram_parameter("yTb", [H, B_shard], BF16, isOutput=False)
    zTb = nc.declare_dram_parameter("zTb", [H, B_shard], BF16, isOutput=False)
    dtr = nc.declare_dram_parameter("dtr", [1, B_shard], F32, isOutput=False)
    # packed stationary: [jt, p, kt2*256 + i*128 + m] (pair-blocked lhsT)
    Wd2 = nc.declare_dram_parameter("Wd2", [NJT, P, K + H], F8, isOutput=False)
    Wy = nc.declare_dram_parameter("Wy", [NJT, P, K + H], F8, isOutput=False)
    Wd1 = nc.declare_dram_parameter("Wd1", [NJT, P, K + H], F8, isOutput=False)
    Wg3 = nc.declare_dram_parameter("Wg3", [NJT, P, K + H], F8, isOutput=False)
    # last two columns: row 0 holds b_dt[0], b_dt[1]
    biasP = nc.declare_dram_parameter("biasP", [P, 4 * NJT + 2], F32, isOutput=False)

    y_newT = nc.declare_dram_parameter("y_newT", [H, B_shard], F32, isOutput=True)
    z_newT = nc.declare_dram_parameter("z_newT", [H, B_shard], F32, isOutput=True)

    with tile.TileContext(nc) as tc, ExitStack() as ctx:
        cpool = ctx.enter_context(tc.tile_pool(name="cpool", bufs=1))
        xpool = ctx.enter_context(tc.tile_pool(name="xpool", bufs=NKT2))
        ypool = ctx.enter_context(tc.tile_pool(name="ypool", bufs=NHT2))
        ybpool = ctx.enter_context(tc.tile_pool(name="ybpool", bufs=NJT))
        zpool = ctx.enter_context(tc.tile_pool(name="zpool", bufs=4))
        znpool = ctx.enter_context(tc.tile_pool(name="znpool", bufs=NHT2))
        wpool = ctx.enter_context(tc.tile_pool(name="wpool", bufs=w_bufs))
        apool = ctx.enter_context(tc.tile_pool(name="apool", bufs=4))
        dpool = ctx.enter_context(tc.tile_pool(name="dpool", bufs=6))
        opool = ctx.enter_context(tc.tile_pool(name="opool", bufs=4))
        bcpool = ctx.enter_context(tc.tile_pool(name="bcpool", bufs=1))
        rpool = ctx.enter_context(tc.tile_pool(name="rpool", bufs=2))
        pspool = ctx.enter_context(tc.tile_pool(name="pspool", bufs=ps_bufs, space="PSUM"))

        bias_sb = cpool.tile([P, 4 * NJT + 2], F32, name="bias_sb")

        def bias_ap(g, jt):
            i = g * NJT + jt
            return bias_sb[:, i : i + 1]

        def col(c, n=1):
            return slice(c * chunk, (c + n) * chunk)

        dt_sb = rpool.tile([1, B_shard], F32, name="dt_sb", tag="dtr", bufs=1)

        def load_w(Wsrc, jt, name, eng, eng2=None):
            w_sb = wpool.tile([P, NPAIR, 2, P], F8, name=name, tag="w")
            if eng2 is None:
                eng.dma_start(
                    w_sb[:],
                    Wsrc[jt].rearrange("p (kt2 i m) -> p kt2 i m", kt2=NPAIR, i=2),
                )
            else:
                h = NPAIR // 2
                hw_ = h * 2 * P
                eng.dma_start(
                    w_sb[:, :h, :, :],
                    Wsrc[jt][:, 0:hw_].rearrange("p (kt2 i m) -> p kt2 i m", kt2=h, i=2),
                )
                eng2.dma_start(
                    w_sb[:, h:, :, :],
                    Wsrc[jt][:, hw_:].rearrange("p (kt2 i m) -> p kt2 i m", kt2=h, i=2),
                )
            return w_sb

        # resident chunk-blocked moving tiles [P, c, i, chunk]
        x3 = [
            xpool.tile([P, nch, 2, chunk], F8, name="x3", tag="xt")
            for _ in range(NKT2)
        ]
        y3 = [
            ypool.tile([P, nch, 2, chunk], F8, name="y3", tag="yt")
            for _ in range(NHT2)
        ]

        # cold-start criticals first: the very first accum group reads
        # x3[0]c0 then wd2_0 (halved over two queues), wy_0 right behind
        nc.sync.dma_start(
            x3[0][:, 0, :, :], xT8[0, 0].rearrange("p (i b) -> p i b", i=2)
        )
        wd2_0 = load_w(Wd2, 0, "wd2_sb", nc.scalar, nc.gpsimd)
        wy_0 = load_w(Wy, 0, "wy_sb", nc.scalar, nc.gpsimd)
        nc.sync.dma_start(bias_sb[:], biasP[:, :])
        nc.sync.dma_start(dt_sb[:], dtr[0:1, :])

        # remaining chunk-column slices round-robined over the 3 DMA queues
        qs = [nc.sync, nc.scalar, nc.gpsimd]
        qi = 0
        for c in range(nch):
            for kt2 in range(NKT2):
                if c == 0 and kt2 == 0:
                    pass
                else:
                    qs[qi % 3].dma_start(
                        x3[kt2][:, c, :, :],
                        xT8[kt2, c].rearrange("p (i b) -> p i b", i=2),
                    )
                    qi += 1
                qs[qi % 3].dma_start(
                    y3[kt2][:, c, :, :],
                    yT8[kt2, c].rearrange("p (i b) -> p i b", i=2),
                )
                qi += 1

        # z_new fp8 chunk-blocked tiles, filled during phase B
        zn3 = []
        for ht2 in range(NHT2):
            zn3.append(znpool.tile([P, nch, 2, chunk], F8, name="zn3", tag="zn"))

        yb = [None] * NJT

        # per-batch dt gates AFTER the input-DMA pushes: these engine ops
        # chain on the dt/bias DMAs and would otherwise block the scalar /
        # gpsimd queue pushes of the x/y flood at stream head
        sg1 = rpool.tile([1, B_shard], F32, name="sg1", tag="sg")
        nc.scalar.activation(
            sg1[:], dt_sb[:], AF.Sigmoid,
            bias=bias_sb[0:1, 4 * NJT : 4 * NJT + 1], scale=wdt00,
        )
        sg2 = rpool.tile([1, B_shard], F32, name="sg2", tag="sg")
        nc.scalar.activation(
            sg2[:], dt_sb[:], AF.Sigmoid,
            bias=bias_sb[0:1, 4 * NJT + 1 : 4 * NJT + 2], scale=wdt10,
        )
        bc1 = bcpool.tile([P, B_shard], F32, name="bc1", tag="bc1")
        nc.gpsimd.partition_broadcast(bc1[:], sg1[0:1, :])
        bc2 = bcpool.tile([P, B_shard], F32, name="bc2", tag="bc2")
        nc.gpsimd.partition_broadcast(bc2[:], sg2[0:1, :])

        def accum_group(ps, w_sb, c, rhs_b):
            """8 DoubleRow matmuls: K+H = 2048-deep contraction."""
            for kt2 in range(NKT2):
                nc.tensor.matmul(
                    ps[:],
                    lhsT=w_sb[:, kt2, :, :],
                    rhs=x3[kt2][:, c, :, :],
                    start=(kt2 == 0),
                    stop=False,
                    perf_mode=DR,
                )
            for kt2 in range(NHT2):
                nc.tensor.matmul(
                    ps[:],
                    lhsT=w_sb[:, NKT2 + kt2, :, :],
                    rhs=rhs_b[kt2][:, c, :, :],
                    start=False,
                    stop=(kt2 == NHT2 - 1),
                    perf_mode=DR,
                )

        # ---- phase B: d2 + y gates -> z_new ----
        wd1_0 = wg3_0 = None
        for jt in range(NJT):
            if jt == 0:
                wd2_sb, wy_sb = wd2_0, wy_0
            else:
                wd2_sb = load_w(Wd2, jt, "wd2_sb", nc.sync)
                wy_sb = load_w(Wy, jt, "wy_sb", nc.scalar)
            # bf16 combine copy of y (phase C consumer) trickles in on gpsimd
            ybt = ybpool.tile([P, B_shard], BF16, name="yb", tag="yb")
            nc.gpsimd.dma_start(ybt[:], yTb[jt * P : (jt + 1) * P, :])
            yb[jt] = ybt
            if jt == NJT - 1:
                # prestage phase C's first weights so the B->C handoff is clean
                wd1_0 = load_w(Wd1, 0, "wd1_sb", nc.sync)
                wg3_0 = load_w(Wg3, 0, "wg3_sb", nc.scalar)
            for c in range(nch):
                cs = col(c)
                zb_sb = zpool.tile([P, chunk], BF16, name="zb_sb", tag="z")
                nc.sync.dma_start(zb_sb[:], zTb[jt * P : (jt + 1) * P, cs])

                ps1 = pspool.tile([P, chunk], F32, name="ps1", tag="ps")
                accum_group(ps1, wd2_sb, c, y3)
                s2 = apool.tile([P, chunk], F32, name="s2", tag="act")
                nc.scalar.activation(s2[:], ps1[:], AF.Sigmoid, bias=bias_ap(0, jt), scale=descale)

                ps2 = pspool.tile([P, chunk], F32, name="ps2", tag="ps")
                accum_group(ps2, wy_sb, c, y3)
                tz = apool.tile([P, chunk], F32, name="tz", tag="act")
                nc.scalar.activation(tz[:], ps2[:], AF.Tanh, bias=bias_ap(1, jt), scale=descale)

                ms2 = dpool.tile([P, chunk], F32, name="ms2", tag="dve")
                nc.vector.tensor_mul(ms2[:], s2[:], bc2[:, cs])
                dlt = dpool.tile([P, chunk], F32, name="dlt", tag="dve")
                nc.vector.tensor_sub(dlt[:], tz[:], zb_sb[:])
                prd = dpool.tile([P, chunk], F32, name="prd", tag="dve")
                nc.vector.tensor_mul(prd[:], ms2[:], dlt[:])
                znc = opool.tile([P, chunk], F32, name="znc", tag="znc")
                nc.vector.tensor_add(znc[:], prd[:], zb_sb[:])
                eng = nc.gpsimd if (c % 2 == 0) else nc.scalar
                eng.dma_start(z_newT[jt * P : (jt + 1) * P, cs], znc[:])
                # fp8 rounding cast into the chunk-blocked tile for GEMM3
                nc.scalar.activation(
                    zn3[jt // 2][:, c, jt % 2, :], znc[:], AF.Copy, bias=0.0, scale=1.0
                )

        # ---- phase C: d1 gate + (i_z + z_new @ W_z.T) -> y_new ----
        for jt in range(NJT):
            if jt == 0:
                wd1_sb, wg3_sb = wd1_0, wg3_0
            else:
                wd1_sb = load_w(Wd1, jt, "wd1_sb", nc.sync)
                wg3_sb = load_w(Wg3, jt, "wg3_sb", nc.scalar)
            for c in range(nch):
                cs = col(c)
                ps3 = pspool.tile([P, chunk], F32, name="ps3", tag="ps")
                accum_group(ps3, wd1_sb, c, y3)
                s1 = apool.tile([P, chunk], F32, name="s1", tag="act")
                nc.scalar.activation(s1[:], ps3[:], AF.Sigmoid, bias=bias_ap(2, jt), scale=descale)

                ps4 = pspool.tile([P, chunk], F32, name="ps4", tag="ps")
                accum_group(ps4, wg3_sb, c, zn3)
                u = apool.tile([P, chunk], F32, name="u", tag="act")
                nc.scalar.activation(u[:], ps4[:], AF.Tanh, bias=bias_ap(3, jt), scale=descale)

                # yn = y + ms1*(u - y)
                ms1 = dpool.tile([P, chunk], F32, name="ms1", tag="dve")
                nc.vector.tensor_mul(ms1[:], s1[:], bc1[:, cs])
                du = dpool.tile([P, chunk], F32, name="du", tag="dve")
                nc.vector.tensor_sub(du[:], u[:], yb[jt][:, cs])
                mu = dpool.tile([P, chunk], F32, name="mu", tag="dve")
                nc.vector.tensor_mul(mu[:], ms1[:], du[:])
                yn = opool.tile([P, chunk], F32, name="yn", tag="yn")
                nc.vector.tensor_add(yn[:], mu[:], yb[jt][:, cs])
                eng = nc.sync if (c % 2 == 0) else nc.gpsimd
                eng.dma_start(y_newT[jt * P : (jt + 1) * P, cs], yn[:])

    nc.compile()
    return nc


def _pack_pair(Wa, Wb):
    """Pair-blocked lhsT packing of two row-major [out, in] weights, fp8:
    pack[jt, p, kt2*256 + i*128 + m] = q8(Wcat*S)[jt*128+m, kt2*256+i*128+p]."""
    Wc = np.concatenate([Wa, Wb], axis=1) * WSCALE
    W8 = Wc.astype(E4NP)
    O, I = W8.shape
    njt, nkt2 = O // P, I // (2 * P)
    return np.ascontiguousarray(
        np.asarray(W8)
        .reshape(njt, P, nkt2, 2, P)
        .transpose(0, 4, 2, 3, 1)
        .reshape(njt, P, I)
    )


def pack_host_inputs(x, y, z, dt, W_ih, b_ih, W_hh, b_hh, W_z, b_z, b_dt, n_cores):
    """Shard batch across cores; pre-transpose + quantize activations; pack weights."""
    B, K = x.shape
    H = y.shape[1]
    NJT = H // P
    Bs = B // n_cores

    xT8 = np.ascontiguousarray(x.T.astype(E4NP))
    ybf = y.astype(BFNP)
    yT8 = np.ascontiguousarray(ybf.astype(np.float32).astype(E4NP).T)
    yTb = np.ascontiguousarray(ybf.T)
    zTb = np.ascontiguousarray(z.astype(BFNP).T)
    dtrow = np.ascontiguousarray(dt.reshape(1, B))

    def chunk_block(aT8, Bs, chunk):
        """[F, Bs] -> [kt2, c, p, i*chunk+b] per-core chunk-blocked fp8."""
        F = aT8.shape[0]
        nkt2 = F // (2 * P)
        nch = Bs // chunk
        return np.ascontiguousarray(
            np.asarray(aT8)
            .reshape(nkt2, 2, P, nch, chunk)  # [kt2, i, p, c, b]
            .transpose(0, 3, 2, 1, 4)         # [kt2, c, p, i, b]
            .reshape(nkt2, nch, P, 2 * chunk)
        )

    Wd2 = _pack_pair(W_ih[H : 2 * H], W_hh[H : 2 * H])
    Wy = _pack_pair(W_ih[3 * H : 4 * H], W_hh[2 * H : 3 * H])
    Wd1 = _pack_pair(W_ih[0:H], W_hh[0:H])
    Wg3 = _pack_pair(W_ih[2 * H : 3 * H], W_z)

    def bias_cols(bvec):
        return bvec.reshape(NJT, P).T  # [P, NJT]

    bdt_cols = np.zeros((P, 2), np.float32)
    bdt_cols[0, 0] = b_dt[0]
    bdt_cols[0, 1] = b_dt[1]
    biasP = np.ascontiguousarray(
        np.concatenate(
            [
                bias_cols(b_ih[H : 2 * H] + b_hh[H : 2 * H]),
                bias_cols(b_ih[3 * H : 4 * H] + b_hh[2 * H : 3 * H]),
                bias_cols(b_ih[0:H] + b_hh[0:H]),
                bias_cols(b_ih[2 * H : 3 * H] + b_z),
                bdt_cols,
            ],
            axis=1,
        ),
        dtype=np.float32,
    )

    in_maps = []
    for c in range(n_cores):
        cs = slice(c * Bs, (c + 1) * Bs)
        in_maps.append(
            {
                "xT8": chunk_block(xT8[:, cs], Bs, 512),
                "yT8": chunk_block(yT8[:, cs], Bs, 512),
                "yTb": np.ascontiguousarray(yTb[:, cs]),
                "zTb": np.ascontiguousarray(zTb[:, cs]),
                "dtr": np.ascontiguousarray(dtrow[:, cs]),
                "Wd2": Wd2,
                "Wy": Wy,
                "Wd1": Wd1,
                "Wg3": Wg3,
                "biasP": biasP,
            }
        )
    return in_maps


def kernel(x, y, z, dt, W_ih, b_ih, W_hh, b_hh, W_z, b_z, W_dt, b_dt):
    x = np.asarray(x, np.float32)
    y = np.asarray(y, np.float32)
    z = np.asarray(z, np.float32)
    dt = np.asarray(dt, np.float32)
    W_ih = np.asarray(W_ih, np.float32)
    b_ih = np.asarray(b_ih, np.float32)
    W_hh = np.asarray(W_hh, np.float32)
    b_hh = np.asarray(b_hh, np.float32)
    W_z = np.asarray(W_z, np.float32)
    b_z = np.asarray(b_z, np.float32)
    W_dt = np.asarray(W_dt, np.float32)
    b_dt = np.asarray(b_dt, np.float32)

    B, K = x.shape
    H = y.shape[1]
    Bs = B // N_CORES

    in_maps = pack_host_inputs(
        x, y, z, dt, W_ih, b_ih, W_hh, b_hh, W_z, b_z, b_dt, N_CORES
    )
    nc = build_nc(
        K,
        H,
        Bs,
        chunk=512,
        wdt00=float(W_dt[0, 0]),
        wdt10=float(W_dt[1, 0]),
    )
    import os

    trace = os.environ.get("LEM_TRACE", "0") == "1"
    tmpdir = os.environ.get("LEM_TMPDIR") or None
    res = run_bass_kernel_spmd(
        nc, in_maps, list(range(N_CORES)), trace=trace, tmpdir=tmpdir
    )
    global LAST_RESULTS
    LAST_RESULTS = res
    y_newT = np.concatenate([r["y_newT"] for r in res.results], axis=1)
    z_newT = np.concatenate([r["z_newT"] for r in res.results], axis=1)
    return (
        np.ascontiguousarray(y_newT.T, dtype=np.float32),
        np.ascontiguousarray(z_newT.T, dtype=np.float32),
    )
